# revision 5
# baseline (speedup 1.0000x reference)
"""Trainium2 Bass kernel for the MCAN-style dual-stream transformer block.

Strategy: data-parallel over batch (B=8 -> 1 batch element per NeuronCore).
All activations flow feature-major [128, H/128, S] in SBUF. FFN + LN in
fp32r, attention GEMMs in bf16. Softmax uses the no-max-subtract form with
the mask folded into the exp activation bias and a ones-column in the V
projection to produce the denominator Z inside the AV matmul.
"""
import numpy as np

P = 128
S = 512
H = 1024
HC = H // P          # 8
SB = S // P          # 4
NH = 16
DH = 64
FF = 4096
FC = FF // P         # 32
NCORES = 8
LN_EPS = 1e-6

_CACHE = {}


def _build_nc():
    import concourse.mybir as mybir
    import concourse.tile as tile
    from concourse import bacc
    from concourse.bass import ts, ds

    F32 = mybir.dt.float32
    DT = mybir.dt.float32r
    BF = mybir.dt.bfloat16
    Alu = mybir.AluOpType
    Act = mybir.ActivationFunctionType

    nc = bacc.Bacc("TRN2", target_bir_lowering=False, debug=False,
                   num_devices=NCORES)

    def din(name, shape, dt):
        return nc.dram_tensor(name, shape, dt, kind="ExternalInput").ap()

    # per-core activations (feature-major)
    x_r = din("x_r", [P, HC, S], F32)      # residual master, streamed
    y_r = din("y_r", [P, HC, S], F32)
    x_b = din("x_b", [P, HC, S], BF)       # GEMM input copies
    y_b = din("y_b", [P, HC, S], BF)
    xp_b = din("xp_b", [P, HC, S], BF)
    yp_b = din("yp_b", [P, HC, S], BF)
    xmb = din("xmb", [P, SB], F32)         # -1e9 where masked, keyed by key pos
    ymb = din("ymb", [P, SB], F32)
    ones128 = din("ones128", [P, 1], DT)
    ones_row = din("ones_row", [1, P], DT)
    ones64_bf = din("ones64_bf", [P, SB * NH], BF)

    mha_w = {}
    for m in ("a", "b", "c"):
        mha_w[m] = dict(
            wk=din(f"wk_{m}", [HC, P, H], BF),        # oc-blocked lhsT
            wqr=din(f"wqr_{m}", [HC, P, 2 * H], BF),  # concat Wq;Wr
            bqr=din(f"bqr_{m}", [P, HC], F32),
            wv=din(f"wv_{m}", [P, HC, H], BF),        # rhs layout
            wm=din(f"wm_{m}", [HC, P, H], BF),
            bm=din(f"bm_{m}", [P, HC], F32),          # bv@Wm + bm folded
        )
    ffn_w = {}
    for f in ("x", "y"):
        ffn_w[f] = dict(
            w1=din(f"w1_{f}", [FC, P, H], DT),
            b1=din(f"b1_{f}", [P, FC], F32),
            w2=din(f"w2_{f}", [HC, P, FF], DT),
            b2=din(f"b2_{f}", [P, HC], F32),
        )
    ln_d = []
    for i in range(6):
        ln_d.append(dict(
            g=din(f"ln{i}_g", [P, HC], F32),
            b=din(f"ln{i}_b", [P, HC], F32),
        ))
    ox = nc.dram_tensor("ox", [P, HC, S], F32, kind="ExternalOutput").ap()
    oy = nc.dram_tensor("oy", [P, HC, S], F32, kind="ExternalOutput").ap()

    from contextlib import ExitStack
    with tile.TileContext(nc) as tc, ExitStack() as top:
        cpool = top.enter_context(tc.tile_pool(name="consts", bufs=1))
        rpool = top.enter_context(tc.tile_pool(name="rows", bufs=1))
        psum = top.enter_context(tc.tile_pool(name="psum", bufs=1, space="PSUM"))

        # ---- constants ----
        t_ones128 = cpool.tile([P, 1], DT, tag="ones128")
        nc.sync.dma_start(t_ones128[:], ones128)
        t_ones_row = cpool.tile([1, P], DT, tag="ones_row")
        nc.sync.dma_start(t_ones_row[:], ones_row)
        t_ones64 = cpool.tile([P, SB * NH], BF, tag="ones64")
        nc.sync.dma_start(t_ones64[:], ones64_bf)
        # packed small f32 constants: [P, 256]
        ck = cpool.tile([P, 256], F32, tag="cpack")
        cslot = [0]
        cmap = {}

        def cload(name, dram_ap, w):
            o = cslot[0]
            nc.sync.dma_start(ck[:, o:o + w], dram_ap)
            cmap[name] = (o, w)
            cslot[0] = o + w

        cload("xmb", xmb, SB)
        cload("ymb", ymb, SB)
        for m in ("a", "b", "c"):
            cload(f"bqr_{m}", mha_w[m]["bqr"], HC)
            cload(f"bm_{m}", mha_w[m]["bm"], HC)
        for f in ("x", "y"):
            cload(f"b1_{f}", ffn_w[f]["b1"], FC)
            cload(f"b2_{f}", ffn_w[f]["b2"], HC)
        for i in range(6):
            cload(f"g{i}", ln_d[i]["g"], HC)
            cload(f"b{i}", ln_d[i]["b"], HC)

        def cc(name, j):
            o, w = cmap[name]
            assert j < w
            return ck[:, o + j:o + j + 1]

        # rows scratch: chained LN stats -> pack into one tile (serial anyway)
        rows_f = rpool.tile([1, 3 * S], F32, tag="rows_f")
        rows_r = rpool.tile([1, 2 * S], DT, tag="rows_r")
        t_iz = rpool.tile([1, S], DT, tag="iz")

        # ---------------- layer norm ----------------
        def layer_norm(spool, s_pre, li, outs):
            """s_pre: [P, HC, S] f32r tile. outs: list of (kind, target, dtype):
            kind 'tile' -> STT writes target[:, c, :]; 'dram' -> staged DMA."""
            ps_sum = psum.tile([1, S], F32, tag="pps", bufs=2)
            ps_sq = psum.tile([1, S], F32, tag="pps", bufs=2)
            for c in range(HC):
                sq = spool.tile([P, S], DT, tag="sq", bufs=1)
                nc.scalar.activation(sq[:], s_pre[:, c, :].bitcast(F32),
                                     Act.Square)
                nc.tensor.matmul(ps_sum[:], t_ones128[:], s_pre[:, c, :],
                                 start=(c == 0), stop=(c == HC - 1))
                nc.tensor.matmul(ps_sq[:], t_ones128[:], sq[:],
                                 start=(c == 0), stop=(c == HC - 1))
            t1 = rows_f[:, 0:S]
            v = rows_f[:, S:2 * S]
            ss = rows_f[:, 2 * S:3 * S]
            nc.scalar.copy(ss, ps_sum[:])
            nc.vector.tensor_mul(t1, ss, ss)
            nc.vector.scalar_tensor_tensor(v, t1, -1.0 / H, ps_sq[:],
                                           op0=Alu.mult, op1=Alu.add)
            nc.scalar.activation(t1, v, Act.Sqrt, scale=1.0 / (H - 1))
            nc.vector.tensor_scalar_add(v, t1, LN_EPS)
            r = rows_r[:, 0:S]
            m = rows_r[:, S:2 * S]
            with nc.allow_low_precision(reason="fp32r rstd is intentional"):
                nc.vector.reciprocal(r, v)
            nc.vector.tensor_scalar_mul(m, ss, 1.0 / H)
            ps_R = psum.tile([P, S], F32, tag="ppz", bufs=1)
            nc.tensor.matmul(ps_R[:], t_ones_row[:], r, start=True, stop=True)
            ps_M = psum.tile([P, S], F32, tag="ppu", bufs=2)
            nc.tensor.matmul(ps_M[:], t_ones_row[:], m, start=True, stop=True)
            for c in range(HC):
                lt = spool.tile([P, S], F32, tag="lt", bufs=2)
                nc.vector.scalar_tensor_tensor(
                    lt[:], s_pre[:, c, :].bitcast(F32), 1.0,
                    ps_M[:], op0=Alu.mult, op1=Alu.subtract)
                nc.vector.scalar_tensor_tensor(
                    lt[:], lt[:], cc(f"g{li}", c),
                    ps_R[:], op0=Alu.mult, op1=Alu.mult)
                for kind, target, odt in outs:
                    if kind == "tile":
                        nc.vector.tensor_scalar_add(
                            target[:, c, :], lt[:], cc(f"b{li}", c))
                    else:
                        st = spool.tile([P, S], F32, tag="ost", bufs=2)
                        nc.vector.tensor_scalar_add(
                            st[:], lt[:], cc(f"b{li}", c))
                        nc.sync.dma_start(target[:, c, :], st[:])

        # ---------------- MHA ----------------
        def mha(mpool, q_b, kv_b, pos_d, mb_name, wkey, resid_d, resid_t,
                spre_t, li, outs):
            """q_b/kv_b: [P,HC,S] bf16 tiles. pos_d: DRAM bf16. resid_d: DRAM
            f32 (streamed) or None with resid_t an SBUF tile. spre_t: [P,HC,S]
            f32r tile for pre-LN sum (may alias resid_t)."""
            W = mha_w[wkey]
            posb = mpool.tile([P, HC, S], BF, tag="posb_att", bufs=1)
            nc.sync.dma_start(posb[:], pos_d)
            kh = mpool.tile([P, HC, S], BF, tag="kh", bufs=1)
            qr = mpool.tile([P, HC, S], BF, tag="qr", bufs=1)
            vh = mpool.tile([P, SB, NH, DH + 1], BF, tag="vh", bufs=1)

            # K projection (no bias: cancels in softmax)
            for oc in range(HC):
                ws = mpool.tile([P, H], BF, tag="wsl", bufs=2)
                nc.sync.dma_start(ws[:], W["wk"][oc])
                pp = psum.tile([P, S], F32, tag="pp", bufs=2)
                for kc in range(HC):
                    nc.tensor.matmul(pp[:], ws[:, ts(kc, P)], kv_b[:, kc, :],
                                     start=(kc == 0), stop=(kc == HC - 1))
                nc.vector.tensor_copy(kh[:, oc, :], pp[:])
            # Q+R projection (fused, bias bq+br)
            for oc in range(HC):
                ws = mpool.tile([P, 2 * H], BF, tag="wsl", bufs=2)
                nc.sync.dma_start(ws[:], W["wqr"][oc])
                pp = psum.tile([P, S], F32, tag="pp", bufs=2)
                for kc in range(2 * HC):
                    rhs = q_b[:, kc, :] if kc < HC else posb[:, kc - HC, :]
                    nc.tensor.matmul(pp[:], ws[:, ts(kc, P)], rhs,
                                     start=(kc == 0), stop=(kc == 2 * HC - 1))
                nc.scalar.activation(qr[:, oc, :], pp[:], Act.Identity,
                                     bias=cc(f"bqr_{wkey}", oc))
            att = mpool.tile([P, HC, S], BF, tag="posb_att", bufs=1)
            # V projection, token-major, with ones column at DH
            nc.vector.tensor_copy(
                vh[:, :, :, DH:DH + 1].rearrange("p a h o -> p (a h o)"),
                t_ones64[:])
            for nt in range(2):
                wsv = mpool.tile([P, HC, S], BF, tag="wsv", bufs=1)
                nc.sync.dma_start(wsv[:], W["wv"][:, :, ds(nt * S, S)])
                for sb in range(SB):
                    pp = psum.tile([P, S], F32, tag="pp", bufs=2)
                    for kc in range(HC):
                        nc.tensor.matmul(pp[:], kv_b[:, kc, ts(sb, P)],
                                         wsv[:, kc, :],
                                         start=(kc == 0), stop=(kc == HC - 1))
                    nc.vector.tensor_copy(
                        vh[:, sb, nt * (NH // 2):(nt + 1) * (NH // 2), 0:DH],
                        pp[:].rearrange("p (h d) -> p h d", d=DH))

            # attention, software-pipelined one head deep
            ets = {}

            def qk_exp(h):
                base = DH * (h % 2)
                c = h // 2
                et = mpool.tile([P, SB, S], BF, tag="expT", bufs=2)
                for kb in range(SB):
                    pps = psum.tile([P, S], F32, tag="pps", bufs=2)
                    nc.tensor.matmul(pps[:],
                                     kh[base:base + DH, c, ts(kb, P)],
                                     qr[base:base + DH, c, :],
                                     start=True, stop=True)
                    nc.scalar.activation(et[:, kb, :], pps[:], Act.Exp,
                                         bias=cc(mb_name, kb), scale=0.125)
                ets[h] = et

            def av_norm(h):
                base = DH * (h % 2)
                c = h // 2
                et = ets.pop(h)
                ppu = psum.tile([DH + 1, S], F32, tag="ppu", bufs=2)
                for kb in range(SB):
                    nc.tensor.matmul(ppu[:], vh[:, kb, h, :], et[:, kb, :],
                                     start=(kb == 0), stop=(kb == SB - 1))
                with nc.allow_low_precision(reason="fp32r invZ is intentional"):
                    nc.vector.reciprocal(t_iz[:], ppu[DH:DH + 1, :])
                ppz = psum.tile([DH, S], F32, tag="ppz", bufs=1)
                nc.tensor.matmul(ppz[:], t_ones_row[:, :DH], t_iz[:],
                                 start=True, stop=True)
                tu = mpool.tile([DH, S], F32, tag="tu", bufs=2)
                nc.scalar.copy(tu[:], ppu[0:DH, :])
                nc.vector.tensor_mul(att[base:base + DH, c, :],
                                     tu[:], ppz[:])

            for h in range(NH + 1):
                if h < NH:
                    qk_exp(h)
                if h >= 1:
                    av_norm(h - 1)

            # merge + bias + residual
            for oc in range(HC):
                ws = mpool.tile([P, H], BF, tag="wsl", bufs=2)
                nc.sync.dma_start(ws[:], W["wm"][oc])
                pp = psum.tile([P, S], F32, tag="pp", bufs=2)
                for kc in range(HC):
                    nc.tensor.matmul(pp[:], ws[:, ts(kc, P)], att[:, kc, :],
                                     start=(kc == 0), stop=(kc == HC - 1))
                if resid_d is not None:
                    rs = mpool.tile([P, S], F32, tag="rsl", bufs=2)
                    nc.sync.dma_start(rs[:], resid_d[:, oc, :])
                    r_in = rs[:]
                else:
                    r_in = resid_t[:, oc, :].bitcast(F32)
                nc.vector.scalar_tensor_tensor(
                    spre_t[:, oc, :], pp[:], cc(f"bm_{wkey}", oc), r_in,
                    op0=Alu.add, op1=Alu.add)
            layer_norm(mpool, spre_t, li, outs)

        # ---------------- FFN ----------------
        def ffn(fpool, in_r, fkey, spre_t, li, outs):
            Wf = ffn_w[fkey]
            hdd = fpool.tile([P, FC, S], DT, tag="hdd", bufs=1)
            for oc in range(FC):
                ws = fpool.tile([P, H], DT, tag="w1s", bufs=3)
                nc.sync.dma_start(ws[:], Wf["w1"][oc])
                pp = psum.tile([P, S], F32, tag="pp", bufs=2)
                for kc in range(HC):
                    nc.tensor.matmul(pp[:], ws[:, ts(kc, P)], in_r[:, kc, :],
                                     start=(kc == 0), stop=(kc == HC - 1))
                nc.scalar.activation(hdd[:, oc, :], pp[:], Act.Relu,
                                     bias=cc(f"b1_{fkey}", oc))
            for oc in range(HC):
                ws = fpool.tile([P, FF], DT, tag="w2s", bufs=2)
                nc.sync.dma_start(ws[:], Wf["w2"][oc])
                pp = psum.tile([P, S], F32, tag="pp", bufs=2)
                for kc in range(FC):
                    nc.tensor.matmul(pp[:], ws[:, ts(kc, P)], hdd[:, kc, :],
                                     start=(kc == 0), stop=(kc == FC - 1))
                nc.vector.scalar_tensor_tensor(
                    spre_t[:, oc, :], pp[:], cc(f"b2_{fkey}", oc),
                    in_r[:, oc, :].bitcast(F32), op0=Alu.add, op1=Alu.add)
            layer_norm(fpool, spre_t, li, outs)

        # ---------------- program ----------------
        with tc.tile_pool(name="acts_late", bufs=1) as alate:
            with tc.tile_pool(name="acts_mid", bufs=1) as amid:
                nx1_r = amid.tile([P, HC, S], DT, tag="nx1_r")
                nx1_b = amid.tile([P, HC, S], BF, tag="nx1_b")
                ny1_r = amid.tile([P, HC, S], DT, tag="ny1_r")
                ny1_b = amid.tile([P, HC, S], BF, tag="ny1_b")
                with tc.tile_pool(name="mtrans", bufs=1) as mt:
                    with tc.tile_pool(name="axy", bufs=1) as axy:
                        t_xb = axy.tile([P, HC, S], BF, tag="xb")
                        nc.sync.dma_start(t_xb[:], x_b)
                        t_yb = axy.tile([P, HC, S], BF, tag="yb")
                        nc.sync.dma_start(t_yb[:], y_b)
                        spre1 = axy.tile([P, HC, S], DT, tag="spre")
                        # S1: new_x = LN0(x + MHA_x2y(v=k=y, q=x, r=xp, m=ym))
                        mha(mt, t_xb, t_yb, xp_b, "ymb", "a", x_r, None,
                            spre1, 0,
                            [("tile", nx1_r, DT), ("tile", nx1_b, BF)])
                        # S2: new_y = LN1(y + MHA_y2x(v=k=x, q=y, r=yp, m=xm))
                        mha(mt, t_yb, t_xb, yp_b, "xmb", "b", y_r, None,
                            spre1, 1,
                            [("tile", ny1_r, DT), ("tile", ny1_b, BF)])
                    nx2 = alate.tile([P, HC, S], DT, tag="nx2")
                    ny2 = alate.tile([P, HC, S], DT, tag="ny2")
                    # S3: new_x = LN2(nx1 + MHA_self(nx1, r=xp, m=xm))
                    mha(mt, nx1_b, nx1_b, xp_b, "xmb", "c", None, nx1_r,
                        nx1_r, 2, [("tile", nx2, DT)])
                    # S4: new_y = LN3(ny1 + MHA_self(ny1, r=yp, m=ym))
                    mha(mt, ny1_b, ny1_b, yp_b, "ymb", "c", None, ny1_r,
                        ny1_r, 3, [("tile", ny2, DT)])
            with tc.tile_pool(name="ftrans", bufs=1) as ft:
                # S5/S6: FFN + final LN -> DRAM
                ffn(ft, nx2, "x", nx2, 4, [("dram", ox, F32)])
                ffn(ft, ny2, "y", ny2, 5, [("dram", oy, F32)])

    nc.compile()
    return nc


def _get_nc():
    if "nc" not in _CACHE:
        _CACHE["nc"] = _build_nc()
    return _CACHE["nc"]


# ---------------- host-side data prep ----------------

def _fm32(t):
    """[S, H] -> feature-major [P, HC, S] f32 contiguous."""
    return np.ascontiguousarray(
        np.asarray(t, np.float32).T.reshape(HC, P, S).transpose(1, 0, 2))


def _fmbf(t):
    import ml_dtypes
    return np.ascontiguousarray(
        np.asarray(t, np.float32).T.reshape(HC, P, S).transpose(1, 0, 2)
        .astype(ml_dtypes.bfloat16))


def _w_oc(W, dt):
    """[din, dout] -> oc-blocked lhsT [dout/P, P, din]."""
    W = np.asarray(W, np.float32)
    din, dout = W.shape
    out = W.reshape(din // P, P, dout // P, P).transpose(2, 1, 0, 3) \
        .reshape(dout // P, P, din)
    return np.ascontiguousarray(out.astype(dt))


def _w_rhs(W, dt):
    """[din, dout] -> rhs layout [P, din/P, dout]."""
    W = np.asarray(W, np.float32)
    din, dout = W.shape
    return np.ascontiguousarray(
        W.reshape(din // P, P, dout).transpose(1, 0, 2).astype(dt))


def _bpp(b, n):
    return np.ascontiguousarray(np.asarray(b, np.float32).reshape(n, P).T)


def _weights_map(params):
    import ml_dtypes
    BFD = ml_dtypes.bfloat16
    m = {}
    for key, name in (("x2y", "a"), ("y2x", "b"), ("xself", "c")):
        p = params[key]
        wq = np.asarray(p["q"]["w"], np.float32)
        wr = np.asarray(p["r"]["w"], np.float32)
        wqr = np.concatenate([wq, wr], axis=0)
        m[f"wk_{name}"] = _w_oc(p["k"]["w"], BFD)
        m[f"wqr_{name}"] = _w_oc(wqr, BFD)
        m[f"bqr_{name}"] = _bpp(np.asarray(p["q"]["b"], np.float32)
                                + np.asarray(p["r"]["b"], np.float32), HC)
        m[f"wv_{name}"] = _w_rhs(p["v"]["w"], BFD)
        m[f"wm_{name}"] = _w_oc(p["merge"]["w"], BFD)
        bmf = (np.asarray(p["v"]["b"], np.float32)
               @ np.asarray(p["merge"]["w"], np.float32)
               + np.asarray(p["merge"]["b"], np.float32))
        m[f"bm_{name}"] = _bpp(bmf, HC)
    for key, name in (("ffnx", "x"), ("ffny", "y")):
        p = params[key]
        m[f"w1_{name}"] = _w_oc(p["fc1"]["w"], np.float32)
        m[f"b1_{name}"] = _bpp(p["fc1"]["b"], FC)
        m[f"w2_{name}"] = _w_oc(p["fc2"]["w"], np.float32)
        m[f"b2_{name}"] = _bpp(p["fc2"]["b"], HC)
    for i, ln in enumerate(params["ln"]):
        g = np.asarray(ln["g"], np.float32)
        b = np.asarray(ln["b"], np.float32)
        m[f"ln{i}_g"] = _bpp(g, HC)
        m[f"ln{i}_b"] = _bpp(b, HC)
    m["ones128"] = np.ones((P, 1), np.float32)
    m["ones_row"] = np.ones((1, P), np.float32)
    m["ones64_bf"] = np.ones((P, SB * NH), BFD)
    return m


def _mask_bias(mask):
    """[1, 1, S] bool -> [P, SB] f32 additive bias on key positions."""
    mb = np.where(np.asarray(mask).reshape(S), np.float32(-1e9),
                  np.float32(0.0))
    return np.ascontiguousarray(mb.reshape(SB, P).T)


def _unfm(t):
    """[P, HC, S] -> [S, H]."""
    return np.ascontiguousarray(t.transpose(1, 0, 2).reshape(H, S).T)


def make_in_maps(x, y, x_mask, y_mask, x_pos, y_pos, params):
    wm = _weights_map(params)
    x = np.asarray(x, np.float32)
    y = np.asarray(y, np.float32)
    x_pos = np.asarray(x_pos, np.float32)
    y_pos = np.asarray(y_pos, np.float32)
    in_maps = []
    for b in range(NCORES):
        im = dict(wm)
        im["x_r"] = _fm32(x[b])
        im["y_r"] = _fm32(y[b])
        im["x_b"] = _fmbf(x[b])
        im["y_b"] = _fmbf(y[b])
        im["xp_b"] = _fmbf(x_pos[b])
        im["yp_b"] = _fmbf(y_pos[b])
        im["xmb"] = _mask_bias(x_mask[b])
        im["ymb"] = _mask_bias(y_mask[b])
        in_maps.append(im)
    return in_maps


def kernel(x, y, x_mask, y_mask, x_pos, y_pos, params):
    from concourse import bass_utils
    nc = _get_nc()
    in_maps = make_in_maps(x, y, x_mask, y_mask, x_pos, y_pos, params)
    res = bass_utils.run_bass_kernel_spmd(nc, in_maps,
                                          core_ids=list(range(NCORES)))
    new_x = np.stack([_unfm(res.results[b]["ox"]) for b in range(NCORES)])
    new_y = np.stack([_unfm(res.results[b]["oy"]) for b in range(NCORES)])
    return (new_x, new_y)
